# revision 13
# baseline (speedup 1.0000x reference)
"""Trainium2 Bass kernel for nn_Attention_75093208203309 (sparse attention).

Contract: kernel(**inputs) takes FULL unsharded inputs (numpy), returns the
FULL [4096, 1024] float32 output. Internally shards query rows across 8
NeuronCores; k/v are computed locally per-core and all-gathered on-device.

Key structural points:
  - boundary_mask is binary, so out = b ? attn_row : x_row is a row SELECT:
    only rows with b=1 (~2048 of 4096) need attention at all. The host packs
    the selected rows into 8*MQ slots (MQ=384 per core, +8 sigma capacity),
    the device computes attention only for those, and the host scatters the
    results back over a copy of x (b=0 rows pass through exactly).
  - All projections run as fp8e4m3 DoubleRow matmuls (weights host-scaled by
    64; the /64 rides the ACT epilogues). Weights are iteration-invariant and
    stay resident in SBUF. k/v are quantized to fp8 before the all-gather.
  - Softmax support reduction: only entries with attention_mask=1 AND
    learnable_mask=1 AND st=1 reach mask level 2, and level-2 entries
    dominate every row, so keep = (conn_logit > -bias) * (am*lm) and
    E = exp(S/32) * keep with exact zeros.
  - S/conn logits and E@v run as fp8 DoubleRow matmuls. keep on DVE (reads
    conn PSUM), exp on ACT (reads S PSUM), the mask multiply split Pool:DVE
    3:1 so no vector engine paces PE.
  - k and v ship in ONE all-gather (concatenated 1 MB fp8 buffer).
  - E@v for d-half 0 runs INSIDE the S loop (psA 2 + psB 2 + O0 3 = 7 PSUM
    banks), deferred so it never chases the Ep tiles still being written.
    d-half 1 runs after, mt-major, so each output tile's normalize+store
    streams while the PE still accumulates later tiles. Row sums (DoubleRow
    vs fp8 ones) are issued first; the 1/rowsum normalize rides the PSUM
    drains as a per-partition activation scale.
"""

import contextlib

import numpy as np
import ml_dtypes  # noqa: F401  (np fp8/bf16 views)

import concourse.bass as bass
import concourse.bacc as bacc
import concourse.mybir as mybir
import concourse.tile as tile
from concourse import bass_utils

f32 = mybir.dt.float32
f32r = mybir.dt.float32r
bf16 = mybir.dt.bfloat16
fp8 = mybir.dt.float8e4
AF = mybir.ActivationFunctionType
ALU = mybir.AluOpType
DR = mybir.MatmulPerfMode.DoubleRow

NCORES = 8
N, D = 4096, 1024
M = N // NCORES          # 512 k/v rows per core
MQ = 384                 # query-row capacity per core (selected rows)
MTQ = MQ // 128          # 3 query m-tiles
G = N // 128             # 32 k-row tiles
GP = G // 2              # 16 k-row tile pairs (DoubleRow E@v)
DC = D // 128            # 8 contraction tiles
WSCALE = 64.0            # fp8 weight pre-scale (avoids e4m3 subnormals)
RG = [list(range(NCORES))]


def build(bias_val: float, timing_mode: bool = False, repeats: int = 1):
    """timing_mode: single-core variant with zk/zv as ExternalInputs and no
    collectives, for TimelineSim cost-model profiling."""
    nc = bacc.Bacc(None, num_devices=NCORES, debug=False)

    xt = nc.dram_tensor("xt", [128, DC, M], fp8, kind="ExternalInput")
    xq = nc.dram_tensor("xq", [128, DC, MQ], fp8, kind="ExternalInput")
    wqt = nc.dram_tensor("wqt", [2, 128, DC, 512], fp8, kind="ExternalInput")
    wkt = nc.dram_tensor("wkt", [2, 128, DC, 512], fp8, kind="ExternalInput")
    wvt = nc.dram_tensor("wvt", [2, 128, DC, 512], fp8, kind="ExternalInput")
    cn = nc.dram_tensor("cn", [2, 128, DC, 512], fp8, kind="ExternalInput")
    pcombo = nc.dram_tensor("pcombo", [128, 2 * DC], f32, kind="ExternalInput")
    bcombo_d = nc.dram_tensor("bcombo", [1, 128 + D], bf16,
                              kind="ExternalInput")
    mmh = nc.dram_tensor("mmh", [G, 128, MQ], mybir.dt.uint8,
                         kind="ExternalInput")
    ones8 = nc.dram_tensor("ones8", [128, 2, 8], mybir.dt.float8e4,
                           kind="ExternalInput")
    out = nc.dram_tensor("out", [MTQ, 128, D], f32, kind="ExternalOutput")

    with tile.TileContext(nc) as tc, contextlib.ExitStack() as ST:
        pp = ST.enter_context(tc.tile_pool(name="persist", bufs=1))
        dp = ST.enter_context(tc.tile_pool(name="dram", bufs=1, space="DRAM"))

        ones_s = pp.tile([128, 2, 8], fp8, name="ones_s")
        pcf = pp.tile([128, 2 * DC], f32, name="pcf")
        bcombo = pp.tile([1, 128 + D], bf16, name="bcombo")
        onesk1 = bcombo[:, 0:128]
        recip_s = pp.tile([128, MTQ], f32, name="recip_s")

        def load_persists():
            nc.sync.dma_start(ones_s[:], ones8.ap())
            nc.sync.dma_start(pcf[:], pcombo.ap())
            nc.sync.dma_start(bcombo[:], bcombo_d.ap())

        # weights are iteration-invariant: load once, keep resident (4 MB)
        wq_h, wk_h, wv_h, cn_h = [
            [pp.tile([128, DC, 512], fp8, name=f"w_{nm}{h}") for h in range(2)]
            for nm in ("q", "k", "v", "c")
        ]

        def load_weights():
            # kT first (k projection runs first); 2 chunks per half so the
            # first matmuls start sooner
            for h in range(2):
                for c in range(2):
                    nc.sync.dma_start(
                        wk_h[h][:, 4 * c : 4 * c + 4, :],
                        wkt.ap()[h][:, 4 * c : 4 * c + 4, :],
                    )
            for wt_d, w_t in ((wvt, wv_h), (wqt, wq_h), (cn, cn_h)):
                for h in range(2):
                    nc.sync.dma_start(w_t[h][:], wt_d.ap()[h])

        if timing_mode:
            zkv = nc.dram_tensor("zkv", [NCORES, 128, 2 * DC * M], fp8,
                                 kind="ExternalInput").ap()

        for _rep in range(repeats):
            # k and v concatenated: one all-gather, one barrier
            kv_loc = dp.tile([128, 2 * DC * M], fp8, name=f"kv_loc{_rep}")
            if not timing_mode:
                zkv = dp.tile([NCORES, 128, 2 * DC * M], fp8, name=f"zkv{_rep}",
                              addr_space="Shared")
            Ep = [
                pp.tile([128, 2, MQ], fp8, tag="Ep", name=f"Ep_{p}_{_rep}",
                        bufs=GP)
                for p in range(GP)
            ]
            # pools whose lifetimes cross phase boundaries, closed manually
            q_stack = contextlib.ExitStack()
            qp = q_stack.enter_context(tc.tile_pool(name="qpool", bufs=1))
            kp = q_stack.enter_context(tc.tile_pool(name="s_kt", bufs=7))
            qt8 = qp.tile([128, DC, MQ], fp8, name="qt8")
            qct8 = qp.tile([128, DC, MQ], fp8, name="qct8")

            ktb_pre = {}

            def load_ktb(j):
                ktb = kp.tile([128, DC, M], fp8, tag="kt", name="ktb")
                nc.sync.dma_start(ktb[:], zkv[j][:, 0 : DC * M])
                ktb_pre[j] = ktb
                return ktb

            # -------- projections k -> v -> q -> conn (fp8 DR) --------
            with (
                tc.tile_pool(name="qkv_x", bufs=1) as xp,
                tc.tile_pool(name="qkv_sb", bufs=2) as sp,
                tc.tile_pool(name="qkv_ps", bufs=8, space="PSUM") as ps1,
            ):
                xt_s = xp.tile([128, DC, M], fp8, name="xt_s")
                xq_s = xp.tile([128, DC, MQ], fp8, name="xq_s")
                # first x tiles load in 2-t chunks so the first matmuls
                # start earlier
                for c in range(4):
                    nc.sync.dma_start(
                        xt_s[:, 2 * c : 2 * c + 2, :],
                        xt.ap()[:, 2 * c : 2 * c + 2, :],
                    )
                nc.sync.dma_start(xq_s[:], xq.ap())

                def mm_half_dr(w_h, rhs8, psums, mm):
                    # fp8 DoubleRow: weights host-scaled by 64; epilogues
                    # divide by 64 on the ACT engine.
                    for tt in range(DC // 2):
                        for oi in range(4):
                            nc.tensor.matmul(
                                psums[oi][:, 0:mm],
                                w_h[:, 2 * tt : 2 * tt + 2,
                                    oi * 128 : (oi + 1) * 128],
                                rhs8[:, 2 * tt : 2 * tt + 2, 0:mm],
                                start=(tt == 0),
                                stop=(tt == DC // 2 - 1),
                                perf_mode=DR,
                            )

                # kT first: it feeds the all-gather.
                if _rep == 0:
                    load_weights()
                    load_persists()
                kpss = []
                for half in range(2):
                    kps = [
                        ps1.tile([128, M], f32, tag="ps1", name=f"kps{half}{i}")
                        for i in range(4)
                    ]
                    mm_half_dr(wk_h[half], xt_s, kps, M)
                    kpss.append(kps)
                kt_sb = sp.tile([128, DC, M], fp8, name="kt_sb", bufs=1)
                for half in range(2):
                    for oi in range(4):
                        ot = half * 4 + oi
                        nc.scalar.activation(
                            kt_sb[:, ot, :], kpss[half][oi][:], AF.Identity,
                            bias=pcf[:, DC + ot : DC + ot + 1],
                            scale=1.0 / WSCALE,
                        )
                    nc.sync.dma_start(
                        kv_loc[:, half * 2048 : (half + 1) * 2048],
                        kt_sb[:, 4 * half : 4 * half + 4, :],
                    )

                # v next: its gather must land by the fused S+O(dh0) loop
                vpss = []
                for dh in range(2):
                    vps = [
                        ps1.tile([128, 512], f32, tag="ps1", name=f"vps{dh}{mt}")
                        for mt in range(M // 128)
                    ]
                    for tt in range(DC // 2):
                        for mt in range(M // 128):
                            nc.tensor.matmul(
                                vps[mt][:],
                                xt_s[:, 2 * tt : 2 * tt + 2,
                                     mt * 128 : (mt + 1) * 128],
                                wv_h[dh][:, 2 * tt : 2 * tt + 2, :],
                                start=(tt == 0),
                                stop=False,
                                perf_mode=DR,
                            )
                    vpss.append(vps)
                v_sb = sp.tile([128, M // 128, D], fp8, name="v_sb", bufs=1)
                for dh in range(2):
                    for mt in range(M // 128):
                        # bv is host-scaled by 64 in bcombo; closes the group
                        nc.tensor.matmul(
                            vpss[dh][mt][:],
                            onesk1,
                            bcombo[:, 128 + dh * 512 : 128 + (dh + 1) * 512],
                            start=False,
                            stop=True,
                        )
                        nc.scalar.activation(
                            v_sb[:, mt, dh * 512 : (dh + 1) * 512],
                            vpss[dh][mt][:], AF.Identity, scale=1.0 / WSCALE,
                        )
                nc.sync.dma_start(kv_loc[:, DC * M : 2 * DC * M], v_sb[:])
                if not timing_mode:
                    nc.gpsimd.collective_compute(
                        "AllGather", ALU.bypass, replica_groups=RG,
                        ins=[kv_loc[:].opt()], outs=[zkv[:].opt()],
                    )
                load_ktb(0)

                # q: feeds the conn projection
                qpss = []
                for half in range(2):
                    qps = [
                        ps1.tile([128, M], f32, tag="ps1", name=f"qps{half}{i}")
                        for i in range(4)
                    ]
                    mm_half_dr(wq_h[half], xq_s, qps, MQ)
                    qpss.append(qps)
                for half in range(2):
                    for oi in range(4):
                        ot = half * 4 + oi
                        nc.scalar.activation(
                            qt8[:, ot, :], qpss[half][oi][:, 0:MQ], AF.Identity,
                            bias=pcf[:, ot : ot + 1],
                            scale=1.0 / WSCALE,
                        )

                for half in range(2):
                    cps = [
                        ps1.tile([128, M], f32, tag="ps1", name=f"cps{half}{i}")
                        for i in range(4)
                    ]
                    mm_half_dr(cn_h[half], qt8, cps, MQ)
                    for oi in range(4):
                        ot = half * 4 + oi
                        # scale-only drain: split ACT/DVE to halve the
                        # serial latency that gates the QKV psum-pool close
                        if oi % 2 == 0:
                            nc.scalar.activation(
                                qct8[:, ot, :], cps[oi][:, 0:MQ], AF.Identity,
                                scale=1.0 / WSCALE,
                            )
                        else:
                            nc.vector.tensor_scalar(
                                qct8[:, ot, :], cps[oi][:, 0:MQ], 1.0 / WSCALE,
                                None, ALU.mult,
                            )

            # v tiles survive into the O phase
            o_stack = contextlib.ExitStack()
            vpool = o_stack.enter_context(
                tc.tile_pool(name="o_v", bufs=3, side="right")
            )
            vt_pre = {}

            def load_vt(j):
                # [128, M//128, D]: all of core j's v rows; serves both d-halves
                vt = vpool.tile([128, M // 128, D], fp8, tag="v", name="vt",
                                bufs=8)
                nc.sync.dma_start(
                    vt[:], zkv[j][:, DC * M : 2 * DC * M]
                )
                vt_pre[j] = vt
                return vt

            # ------- fused S + O(dh0) phase: logits, keep, exp, E@v-half0 -------
            # PSUM budget: psA(2) + psB(2) + O_dh0(3) = 7 banks, so the
            # E@v accumulation for d-half 0 proceeds DURING the S loop
            # instead of serializing behind it.
            o_stack2 = contextlib.ExitStack()
            psO0 = o_stack2.enter_context(
                tc.tile_pool(name="o_ps0", bufs=1, space="PSUM"))
            O_ps0 = [
                psO0.tile([128, 512], f32, tag="O0", name=f"O0_{mt}", bufs=MTQ)
                for mt in range(MTQ)
            ]
            with (
                tc.tile_pool(name="s_m", bufs=6) as mp,
                tc.tile_pool(name="s_t", bufs=12) as tpool,
                tc.tile_pool(name="s_psA", bufs=2, space="PSUM") as psA,
                tc.tile_pool(name="s_psB", bufs=2, space="PSUM") as psB,
            ):
                # interleave ktb/vt prefetch: the SP DMA queue is FIFO, so
                # bulk-issuing all vt loads would delay the critical ktb
                # stream by ~12us of transfers
                load_ktb(1)
                load_vt(0)
                for j in range(NCORES):
                    if j + 2 < NCORES:
                        load_ktb(j + 2)
                    if j + 1 < NCORES:
                        load_vt(j + 1)
                    ktb = ktb_pre.pop(j, None) or load_ktb(j)
                    ktb_pre.pop(j, None)
                    mm_t = mp.tile([128, 4, MQ], mybir.dt.uint8, tag="mm",
                                   name="mm_t")
                    nc.sync.dma_start(
                        mm_t[:],
                        mmh.ap()[4 * j : 4 * j + 4]
                        .rearrange("g p m -> p g m"),
                    )
                    for pb in range(2):
                        for gi2 in range(2):
                            gi = 2 * pb + gi2
                            g = j * 4 + gi
                            B = psB.tile([128, MQ], f32, tag="B", name="Bps")
                            A = psA.tile([128, MQ], f32, tag="A", name="Aps")
                            # interleaved B/A pairs share lhsT
                            for tt in range(DC // 2):
                                lhsT = ktb[:, 2 * tt : 2 * tt + 2,
                                           gi * 128 : (gi + 1) * 128]
                                nc.tensor.matmul(
                                    B[:], lhsT, qt8[:, 2 * tt : 2 * tt + 2, :],
                                    start=(tt == 0), stop=(tt == DC // 2 - 1),
                                    perf_mode=DR,
                                )
                                nc.tensor.matmul(
                                    A[:], lhsT, qct8[:, 2 * tt : 2 * tt + 2, :],
                                    start=(tt == 0), stop=(tt == DC // 2 - 1),
                                    perf_mode=DR,
                                )
                            keep = tpool.tile([128, MQ], f32, tag="keep",
                                              name="keep")
                            nc.vector.scalar_tensor_tensor(
                                keep[:], A[:], -bias_val, mm_t[:, gi, :],
                                ALU.is_gt, ALU.mult,
                            )
                            e1 = tpool.tile([128, MQ], bf16, tag="e1", name="e1")
                            nc.scalar.activation(
                                e1[:], B[:], AF.Exp, scale=1.0 / 32.0
                            )
                            # Masked entries become exact 0 in Ep. Pool
                            # (GPSIMD) can't read PSUM, so e1/keep are SBUF;
                            # every 4th tile runs on DVE so neither vector
                            # engine paces the S loop above PE.
                            gp, ep_i = divmod(g, 2)
                            eng = nc.vector if g % 4 == 3 else nc.gpsimd
                            eng.tensor_tensor(
                                Ep[gp][:, ep_i, :], e1[:], keep[:], ALU.mult
                            )
                    # E@v d-half 0, deferred so the O matmuls never chase
                    # the Ep pairs still being written by Pool/DVE
                    for jo in ([j - 3] if j >= 3 else []) + (
                        [j - 2, j - 1, j] if j == NCORES - 1 else []
                    ):
                        vt = vt_pre[jo]
                        for b in range(2):
                            p = 2 * jo + b
                            for mt in range(MTQ):
                                nc.tensor.matmul(
                                    O_ps0[mt][:],
                                    Ep[p][:, :, mt * 128 : (mt + 1) * 128],
                                    vt[:, 2 * b : 2 * b + 2, 0:512],
                                    start=(p == 0),
                                    stop=(p == GP - 1),
                                    perf_mode=DR,
                                )
            q_stack.close()  # qt/qct + ktb SBUF released before dh1 phase

            # -------- dh1 phase: row sums, E @ v-half1, normalize, store ------
            with (
                tc.tile_pool(name="o_out", bufs=2) as opool,
                tc.tile_pool(name="o_ps", bufs=1, space="PSUM") as psO,
            ):
                O_ps1 = [
                    psO.tile([128, 512], f32, tag="O1", name=f"O1_{mt}",
                             bufs=MTQ)
                    for mt in range(MTQ)
                ]
                S_all = psO.tile([128, MTQ, 8], f32, name="S_all")
                S_ps = [S_all[:, mt, :] for mt in range(MTQ)]
                # sums first: Ep is fully materialized, so the row sums land
                # early; the normalize rides the O drains as an activation
                # scale (1/rowsum per partition).
                for mt in range(MTQ):
                    for p in range(GP):
                        nc.tensor.matmul(
                            S_ps[mt][:],
                            Ep[p][:, :, mt * 128 : (mt + 1) * 128],
                            ones_s[:],
                            start=(p == 0),
                            stop=(p == GP - 1),
                            perf_mode=DR,
                        )
                ot_st = [
                    opool.tile([128, MTQ, 512], f32, tag="ot", name=f"ot_st{dh}")
                    for dh in range(2)
                ]
                for mt in range(MTQ):
                    nc.vector.reciprocal(
                        recip_s[:, mt : mt + 1], S_ps[mt][:, 0:1]
                    )
                    # dh0 normalize+store as soon as its sum lands,
                    # overlapping the O1 matmuls (ACT reads PSUM directly)
                    nc.scalar.activation(
                        ot_st[0][:, mt, :], O_ps0[mt][:], AF.Identity,
                        scale=recip_s[:, mt : mt + 1],
                    )
                    nc.sync.dma_start(
                        out.ap()[mt, :, 0:512], ot_st[0][:, mt, :]
                    )
                # mt-major accumulation: O_ps1[mt] closes after its own 32
                # matmuls, so each normalize+store streams out while the PE
                # still accumulates the later mt tiles.
                for mt in range(MTQ):
                    for j in range(NCORES):
                        vt = vt_pre[j]
                        for b in range(2):
                            p = 2 * j + b
                            nc.tensor.matmul(
                                O_ps1[mt][:],
                                Ep[p][:, :, mt * 128 : (mt + 1) * 128],
                                vt[:, 2 * b : 2 * b + 2, 512:1024],
                                start=(p == 0),
                                stop=(p == GP - 1),
                                perf_mode=DR,
                            )
                    nc.vector.tensor_scalar(
                        ot_st[1][:, mt, :], O_ps1[mt][:],
                        recip_s[:, mt : mt + 1], None, ALU.mult,
                    )
                    nc.sync.dma_start(
                        out.ap()[mt, :, 512:1024], ot_st[1][:, mt, :]
                    )
                for j in range(NCORES):
                    vt_pre.pop(j)
            o_stack2.close()
            o_stack.close()


    nc.compile()
    return nc


def make_in_maps(x, attention_mask, learnable_mask, boundary_mask,
                 W_q, b_q, W_k, b_k, W_v, b_v, connection):
    x = np.asarray(x, np.float32)
    mm_full = (np.asarray(attention_mask, np.float32)
               * np.asarray(learnable_mask, np.float32)).astype(np.uint8)
    boundary = np.asarray(boundary_mask, np.float32).reshape(N)

    sel = np.nonzero(boundary > 0.5)[0]
    cap = NCORES * MQ
    assert len(sel) <= cap, (
        f"boundary selects {len(sel)} rows > capacity {cap}"
    )
    sel_padded = np.concatenate(
        [sel, np.zeros(cap - len(sel), np.int64)])

    def w_halves(wt, dt, scale=1.0):  # wt: [D, D], rows = contraction dim
        # -> [2, 128, DC, 512]: [half][p][t][d] = wt[t*128+p][half*512+d]
        a = np.asarray(wt, np.float32).reshape(DC, 128, 2, 512) * scale
        return np.ascontiguousarray(a.transpose(2, 1, 0, 3)).astype(dt)

    wqt_h = w_halves(np.asarray(W_q, np.float32).T, ml_dtypes.float8_e4m3, WSCALE)
    wkt_h = w_halves(np.asarray(W_k, np.float32).T, ml_dtypes.float8_e4m3, WSCALE)
    wvt_h = w_halves(np.asarray(W_v, np.float32).T, ml_dtypes.float8_e4m3, WSCALE)
    cn_h = w_halves(np.asarray(connection, np.float32), ml_dtypes.float8_e4m3,
                    WSCALE)
    bq_h = np.asarray(b_q, np.float32).reshape(DC, 128).T
    bk_h = np.asarray(b_k, np.float32).reshape(DC, 128).T
    bcombo_h = np.concatenate(
        [np.ones((1, 128), np.float32),
         WSCALE * np.asarray(b_v, np.float32).reshape(1, D)],
        axis=1).astype(ml_dtypes.bfloat16)

    def xpose(rows_x):  # [m, D] -> [128, DC, m]: [p][t][i] = x[i][t*128+p]
        m = rows_x.shape[0]
        return np.ascontiguousarray(
            rows_x.T.reshape(DC, 128, m).transpose(1, 0, 2)).astype(
            ml_dtypes.float8_e4m3)

    in_maps = []
    for c in range(NCORES):
        rows = slice(c * M, (c + 1) * M)
        rows_q = sel_padded[c * MQ : (c + 1) * MQ]
        in_maps.append(dict(
            xt=xpose(x[rows]),
            xq=xpose(x[rows_q]),
            wqt=wqt_h, wkt=wkt_h, wvt=wvt_h, cn=cn_h,
            pcombo=np.ascontiguousarray(
                np.concatenate([bq_h, bk_h], axis=1)),
            bcombo=bcombo_h,
            mmh=np.ascontiguousarray(mm_full[rows_q].T).reshape(G, 128, MQ),
            ones8=np.ones((128, 2, 8), dtype=ml_dtypes.float8_e4m3),
        ))
    return in_maps, sel


_cache = {}


def kernel(x, attention_mask, learnable_mask, boundary_mask,
           W_q, b_q, W_k, b_k, W_v, b_v, connection, bias):
    bias_val = float(np.asarray(bias).reshape(-1)[0])
    if bias_val not in _cache:
        _cache[bias_val] = build(bias_val)
    nc = _cache[bias_val]
    in_maps, sel = make_in_maps(x, attention_mask, learnable_mask,
                                boundary_mask, W_q, b_q, W_k, b_k, W_v, b_v,
                                connection)
    res = bass_utils.run_bass_kernel_spmd(nc, in_maps, core_ids=list(range(NCORES)))
    attn = np.concatenate(
        [res.results[c]["out"].reshape(MQ, D) for c in range(NCORES)], axis=0)
    result = np.array(x, np.float32, copy=True)
    result[sel] = attn[: len(sel)]
    return result


# revision 16
# speedup vs baseline: 1.9163x; 1.9163x over previous
"""Trainium2 Bass kernel for nn_Attention_75093208203309 (sparse attention).

Contract: kernel(**inputs) takes FULL unsharded inputs (numpy), returns the
FULL [4096, 1024] float32 output. Internally shards query rows across 8
NeuronCores; k/v are computed locally per-core and all-gathered on-device.

Key structural points:
  - boundary_mask is binary, so out = b ? attn_row : x_row is a row SELECT:
    only rows with b=1 (~2048 of 4096) need attention at all. The host packs
    the selected rows into 8*MQ slots (MQ=384 per core, +8 sigma capacity),
    the device computes attention only for those, and the host scatters the
    results back over a copy of x (b=0 rows pass through exactly).
  - All projections run as fp8e4m3 DoubleRow matmuls (weights host-scaled by
    64; the /64 rides the ACT epilogues). Weights are iteration-invariant and
    stay resident in SBUF. k/v are quantized to fp8 before the all-gather.
  - Softmax support reduction: only entries with attention_mask=1 AND
    learnable_mask=1 AND st=1 reach mask level 2, and level-2 entries
    dominate every row, so keep = (conn_logit > -bias) * (am*lm) and
    E = exp(S/32) * keep with exact zeros.
  - S/conn logits and E@v run as fp8 DoubleRow matmuls. keep on DVE (reads
    conn PSUM), exp on ACT (reads S PSUM), the mask multiply split Pool:DVE
    3:1 so no vector engine paces PE.
  - k and v ship in ONE all-gather (concatenated 1 MB fp8 buffer).
  - E@v for d-half 0 runs INSIDE the S loop (psA 2 + psB 2 + O0 3 = 7 PSUM
    banks), deferred so it never chases the Ep tiles still being written.
    d-half 1 runs after, mt-major, so each output tile's normalize+store
    streams while the PE still accumulates later tiles. Row sums (DoubleRow
    vs fp8 ones) are issued first; the 1/rowsum normalize rides the PSUM
    drains as a per-partition activation scale.
"""

import contextlib

import numpy as np
import ml_dtypes  # noqa: F401  (np fp8/bf16 views)

import concourse.bass as bass
import concourse.bacc as bacc
import concourse.mybir as mybir
import concourse.tile as tile
from concourse import bass_utils

f32 = mybir.dt.float32
f32r = mybir.dt.float32r
bf16 = mybir.dt.bfloat16
fp8 = mybir.dt.float8e4
AF = mybir.ActivationFunctionType
ALU = mybir.AluOpType
DR = mybir.MatmulPerfMode.DoubleRow

NCORES = 8
N, D = 4096, 1024
M = N // NCORES          # 512 k/v rows per core
MQ = 384                 # query-row capacity per core (selected rows)
MTQ = MQ // 128          # 3 query m-tiles
G = N // 128             # 32 k-row tiles
GP = G // 2              # 16 k-row tile pairs (DoubleRow E@v)
DC = D // 128            # 8 contraction tiles
WSCALE = 64.0            # fp8 weight pre-scale (avoids e4m3 subnormals)
RG = [list(range(NCORES))]


def build(bias_val: float, timing_mode: bool = False, repeats: int = 1):
    """timing_mode: single-core variant with zk/zv as ExternalInputs and no
    collectives, for TimelineSim cost-model profiling."""
    nc = bacc.Bacc(None, num_devices=NCORES, debug=False)

    xt = nc.dram_tensor("xt", [128, DC, M], fp8, kind="ExternalInput")
    xq = nc.dram_tensor("xq", [128, DC, MQ], fp8, kind="ExternalInput")
    wqt = nc.dram_tensor("wqt", [2, 128, DC, 512], fp8, kind="ExternalInput")
    wkt = nc.dram_tensor("wkt", [2, 128, DC, 512], fp8, kind="ExternalInput")
    wvt = nc.dram_tensor("wvt", [2, 128, DC, 512], fp8, kind="ExternalInput")
    cn = nc.dram_tensor("cn", [2, 128, DC, 512], fp8, kind="ExternalInput")
    pcombo = nc.dram_tensor("pcombo", [128, 2 * DC], f32, kind="ExternalInput")
    bcombo_d = nc.dram_tensor("bcombo", [1, 128 + D], bf16,
                              kind="ExternalInput")
    mmh = nc.dram_tensor("mmh", [G, 128, MQ], mybir.dt.uint8,
                         kind="ExternalInput")
    ones8 = nc.dram_tensor("ones8", [128, 2, 8], mybir.dt.float8e4,
                           kind="ExternalInput")
    out = nc.dram_tensor("out", [MTQ, 128, D], f32, kind="ExternalOutput")

    with tile.TileContext(nc) as tc, contextlib.ExitStack() as ST:
        pp = ST.enter_context(tc.tile_pool(name="persist", bufs=1))
        dp = ST.enter_context(tc.tile_pool(name="dram", bufs=1, space="DRAM"))

        ones_s = pp.tile([128, 2, 8], fp8, name="ones_s")
        pcf = pp.tile([128, 2 * DC], f32, name="pcf")
        bcombo = pp.tile([1, 128 + D], bf16, name="bcombo")
        onesk1 = bcombo[:, 0:128]
        recip_s = pp.tile([128, MTQ], f32, name="recip_s")

        def load_persists():
            nc.sync.dma_start(ones_s[:], ones8.ap())
            nc.sync.dma_start(pcf[:], pcombo.ap())
            nc.sync.dma_start(bcombo[:], bcombo_d.ap())

        # weights are iteration-invariant: load once, keep resident (4 MB)
        wq_h, wk_h, wv_h, cn_h = [
            [pp.tile([128, DC, 512], fp8, name=f"w_{nm}{h}") for h in range(2)]
            for nm in ("q", "k", "v", "c")
        ]

        def load_weights():
            # kT first (k projection runs first); 2 chunks per half so the
            # first matmuls start sooner
            for h in range(2):
                for c in range(2):
                    nc.sync.dma_start(
                        wk_h[h][:, 4 * c : 4 * c + 4, :],
                        wkt.ap()[h][:, 4 * c : 4 * c + 4, :],
                    )
            for wt_d, w_t in ((wvt, wv_h), (wqt, wq_h), (cn, cn_h)):
                for h in range(2):
                    nc.sync.dma_start(w_t[h][:], wt_d.ap()[h])

        if timing_mode:
            zkv = nc.dram_tensor("zkv", [NCORES, 128, 2 * DC * M], fp8,
                                 kind="ExternalInput").ap()

        split_cc = not timing_mode
        for _rep in range(repeats):
            if timing_mode:
                zk_ap = lambda j: zkv[j][:, 0 : DC * M]          # noqa: E731
                zv_ap = lambda j: zkv[j][:, DC * M : 2 * DC * M]  # noqa: E731
            else:
                # k and v gathered separately: the k-gather is issued right
                # after the k projection and overlaps v/q/conn + S-loop start
                zk = dp.tile([NCORES, 128, DC * M], fp8, name=f"zk{_rep}",
                             addr_space="Shared")
                zv = dp.tile([NCORES, 128, DC * M], fp8, name=f"zv{_rep}",
                             addr_space="Shared")
                kl = dp.tile([128, DC * M], fp8, name=f"kl{_rep}")
                vl = dp.tile([128, DC * M], fp8, name=f"vl{_rep}")
                zk_ap = lambda j: zk[j][:, :]  # noqa: E731
                zv_ap = lambda j: zv[j][:, :]  # noqa: E731
            Ep = [
                pp.tile([128, 2, MQ], fp8, tag="Ep", name=f"Ep_{p}_{_rep}",
                        bufs=GP)
                for p in range(GP)
            ]
            # pools whose lifetimes cross phase boundaries, closed manually
            q_stack = contextlib.ExitStack()
            qp = q_stack.enter_context(tc.tile_pool(name="qpool", bufs=1))
            kp = q_stack.enter_context(tc.tile_pool(name="s_kt", bufs=7))
            qt8 = qp.tile([128, DC, MQ], fp8, name="qt8")
            qct8 = qp.tile([128, DC, MQ], fp8, name="qct8")

            ktb_pre = {}

            def load_ktb(j):
                ktb = kp.tile([128, DC, M], fp8, tag="kt", name="ktb")
                nc.sync.dma_start(ktb[:], zk_ap(j))
                ktb_pre[j] = ktb
                return ktb

            # -------- projections k -> v -> q -> conn (fp8 DR) --------
            with (
                tc.tile_pool(name="qkv_x", bufs=1) as xp,
                tc.tile_pool(name="qkv_sb", bufs=2) as sp,
                tc.tile_pool(name="qkv_ps", bufs=8, space="PSUM") as ps1,
            ):
                xt_s = xp.tile([128, DC, M], fp8, name="xt_s")
                xq_s = xp.tile([128, DC, MQ], fp8, name="xq_s")
                # first x tiles load in 2-t chunks so the first matmuls
                # start earlier
                for c in range(4):
                    nc.sync.dma_start(
                        xt_s[:, 2 * c : 2 * c + 2, :],
                        xt.ap()[:, 2 * c : 2 * c + 2, :],
                    )
                nc.sync.dma_start(xq_s[:], xq.ap())

                def mm_half_dr(w_h, rhs8, psums, mm):
                    # fp8 DoubleRow: weights host-scaled by 64; epilogues
                    # divide by 64 on the ACT engine.
                    for tt in range(DC // 2):
                        for oi in range(4):
                            nc.tensor.matmul(
                                psums[oi][:, 0:mm],
                                w_h[:, 2 * tt : 2 * tt + 2,
                                    oi * 128 : (oi + 1) * 128],
                                rhs8[:, 2 * tt : 2 * tt + 2, 0:mm],
                                start=(tt == 0),
                                stop=(tt == DC // 2 - 1),
                                perf_mode=DR,
                            )

                # kT first: it feeds the all-gather.
                if _rep == 0:
                    load_weights()
                    load_persists()
                kpss = []
                for half in range(2):
                    kps = [
                        ps1.tile([128, M], f32, tag="ps1", name=f"kps{half}{i}")
                        for i in range(4)
                    ]
                    mm_half_dr(wk_h[half], xt_s, kps, M)
                    kpss.append(kps)
                kt_sb = sp.tile([128, DC, M], fp8, name="kt_sb", bufs=1)
                for half in range(2):
                    for oi in range(4):
                        ot = half * 4 + oi
                        nc.scalar.activation(
                            kt_sb[:, ot, :], kpss[half][oi][:], AF.Identity,
                            bias=pcf[:, DC + ot : DC + ot + 1],
                            scale=1.0 / WSCALE,
                        )
                    nc.sync.dma_start(
                        kv_loc[:, half * 2048 : (half + 1) * 2048],
                        kt_sb[:, 4 * half : 4 * half + 4, :],
                    )

                # v next: its gather must land by the fused S+O(dh0) loop
                vpss = []
                for dh in range(2):
                    vps = [
                        ps1.tile([128, 512], f32, tag="ps1", name=f"vps{dh}{mt}")
                        for mt in range(M // 128)
                    ]
                    for tt in range(DC // 2):
                        for mt in range(M // 128):
                            nc.tensor.matmul(
                                vps[mt][:],
                                xt_s[:, 2 * tt : 2 * tt + 2,
                                     mt * 128 : (mt + 1) * 128],
                                wv_h[dh][:, 2 * tt : 2 * tt + 2, :],
                                start=(tt == 0),
                                stop=False,
                                perf_mode=DR,
                            )
                    vpss.append(vps)
                v_sb = sp.tile([128, M // 128, D], fp8, name="v_sb", bufs=1)
                for dh in range(2):
                    for mt in range(M // 128):
                        # bv is host-scaled by 64 in bcombo; closes the group
                        nc.tensor.matmul(
                            vpss[dh][mt][:],
                            onesk1,
                            bcombo[:, 128 + dh * 512 : 128 + (dh + 1) * 512],
                            start=False,
                            stop=True,
                        )
                        nc.scalar.activation(
                            v_sb[:, mt, dh * 512 : (dh + 1) * 512],
                            vpss[dh][mt][:], AF.Identity, scale=1.0 / WSCALE,
                        )
                nc.sync.dma_start(kv_loc[:, DC * M : 2 * DC * M], v_sb[:])
                if not timing_mode:
                    nc.gpsimd.collective_compute(
                        "AllGather", ALU.bypass, replica_groups=RG,
                        ins=[kv_loc[:].opt()], outs=[zkv[:].opt()],
                    )
                load_ktb(0)

                # q: feeds the conn projection
                qpss = []
                for half in range(2):
                    qps = [
                        ps1.tile([128, M], f32, tag="ps1", name=f"qps{half}{i}")
                        for i in range(4)
                    ]
                    mm_half_dr(wq_h[half], xq_s, qps, MQ)
                    qpss.append(qps)
                for half in range(2):
                    for oi in range(4):
                        ot = half * 4 + oi
                        nc.scalar.activation(
                            qt8[:, ot, :], qpss[half][oi][:, 0:MQ], AF.Identity,
                            bias=pcf[:, ot : ot + 1],
                            scale=1.0 / WSCALE,
                        )

                for half in range(2):
                    cps = [
                        ps1.tile([128, M], f32, tag="ps1", name=f"cps{half}{i}")
                        for i in range(4)
                    ]
                    mm_half_dr(cn_h[half], qt8, cps, MQ)
                    for oi in range(4):
                        ot = half * 4 + oi
                        # scale-only drain: split ACT/DVE to halve the
                        # serial latency that gates the QKV psum-pool close
                        if oi % 2 == 0:
                            nc.scalar.activation(
                                qct8[:, ot, :], cps[oi][:, 0:MQ], AF.Identity,
                                scale=1.0 / WSCALE,
                            )
                        else:
                            nc.vector.tensor_scalar(
                                qct8[:, ot, :], cps[oi][:, 0:MQ], 1.0 / WSCALE,
                                None, ALU.mult,
                            )

            # v tiles survive into the O phase
            o_stack = contextlib.ExitStack()
            vpool = o_stack.enter_context(
                tc.tile_pool(name="o_v", bufs=3, side="right")
            )
            vt_pre = {}

            def load_vt(j):
                # [128, M//128, D]: all of core j's v rows; serves both d-halves
                vt = vpool.tile([128, M // 128, D], fp8, tag="v", name="vt",
                                bufs=8)
                nc.sync.dma_start(
                    vt[:], zkv[j][:, DC * M : 2 * DC * M]
                )
                vt_pre[j] = vt
                return vt

            # ------- fused S + O(dh0) phase: logits, keep, exp, E@v-half0 -------
            # PSUM budget: psA(2) + psB(2) + O_dh0(3) = 7 banks, so the
            # E@v accumulation for d-half 0 proceeds DURING the S loop
            # instead of serializing behind it.
            o_stack2 = contextlib.ExitStack()
            psO0 = o_stack2.enter_context(
                tc.tile_pool(name="o_ps0", bufs=1, space="PSUM"))
            O_ps0 = [
                psO0.tile([128, 512], f32, tag="O0", name=f"O0_{mt}", bufs=MTQ)
                for mt in range(MTQ)
            ]
            with (
                tc.tile_pool(name="s_m", bufs=6) as mp,
                tc.tile_pool(name="s_t", bufs=12) as tpool,
                tc.tile_pool(name="s_psA", bufs=2, space="PSUM") as psA,
                tc.tile_pool(name="s_psB", bufs=2, space="PSUM") as psB,
            ):
                # interleave ktb/vt prefetch: the SP DMA queue is FIFO, so
                # bulk-issuing all vt loads would delay the critical ktb
                # stream by ~12us of transfers
                load_ktb(1)
                load_vt(0)
                for j in range(NCORES):
                    if j + 2 < NCORES:
                        load_ktb(j + 2)
                    if j + 1 < NCORES:
                        load_vt(j + 1)
                    ktb = ktb_pre.pop(j, None) or load_ktb(j)
                    ktb_pre.pop(j, None)
                    mm_t = mp.tile([128, 4, MQ], mybir.dt.uint8, tag="mm",
                                   name="mm_t")
                    nc.sync.dma_start(
                        mm_t[:],
                        mmh.ap()[4 * j : 4 * j + 4]
                        .rearrange("g p m -> p g m"),
                    )
                    for pb in range(2):
                        for gi2 in range(2):
                            gi = 2 * pb + gi2
                            g = j * 4 + gi
                            B = psB.tile([128, MQ], f32, tag="B", name="Bps")
                            A = psA.tile([128, MQ], f32, tag="A", name="Aps")
                            # interleaved B/A pairs share lhsT
                            for tt in range(DC // 2):
                                lhsT = ktb[:, 2 * tt : 2 * tt + 2,
                                           gi * 128 : (gi + 1) * 128]
                                nc.tensor.matmul(
                                    B[:], lhsT, qt8[:, 2 * tt : 2 * tt + 2, :],
                                    start=(tt == 0), stop=(tt == DC // 2 - 1),
                                    perf_mode=DR,
                                )
                                nc.tensor.matmul(
                                    A[:], lhsT, qct8[:, 2 * tt : 2 * tt + 2, :],
                                    start=(tt == 0), stop=(tt == DC // 2 - 1),
                                    perf_mode=DR,
                                )
                            keep = tpool.tile([128, MQ], f32, tag="keep",
                                              name="keep")
                            nc.vector.scalar_tensor_tensor(
                                keep[:], A[:], -bias_val, mm_t[:, gi, :],
                                ALU.is_gt, ALU.mult,
                            )
                            e1 = tpool.tile([128, MQ], bf16, tag="e1", name="e1")
                            nc.scalar.activation(
                                e1[:], B[:], AF.Exp, scale=1.0 / 32.0
                            )
                            # Masked entries become exact 0 in Ep. Pool
                            # (GPSIMD) can't read PSUM, so e1/keep are SBUF;
                            # every 4th tile runs on DVE so neither vector
                            # engine paces the S loop above PE.
                            gp, ep_i = divmod(g, 2)
                            eng = nc.vector if g % 4 == 3 else nc.gpsimd
                            eng.tensor_tensor(
                                Ep[gp][:, ep_i, :], e1[:], keep[:], ALU.mult
                            )
                    # E@v d-half 0, deferred so the O matmuls never chase
                    # the Ep pairs still being written by Pool/DVE
                    for jo in ([j - 3] if j >= 3 else []) + (
                        [j - 2, j - 1, j] if j == NCORES - 1 else []
                    ):
                        vt = vt_pre[jo]
                        for b in range(2):
                            p = 2 * jo + b
                            for mt in range(MTQ):
                                nc.tensor.matmul(
                                    O_ps0[mt][:],
                                    Ep[p][:, :, mt * 128 : (mt + 1) * 128],
                                    vt[:, 2 * b : 2 * b + 2, 0:512],
                                    start=(p == 0),
                                    stop=(p == GP - 1),
                                    perf_mode=DR,
                                )
            q_stack.close()  # qt/qct + ktb SBUF released before dh1 phase

            # -------- dh1 phase: row sums, E @ v-half1, normalize, store ------
            with (
                tc.tile_pool(name="o_out", bufs=2) as opool,
                tc.tile_pool(name="o_ps", bufs=1, space="PSUM") as psO,
            ):
                O_ps1 = [
                    psO.tile([128, 512], f32, tag="O1", name=f"O1_{mt}",
                             bufs=MTQ)
                    for mt in range(MTQ)
                ]
                S_all = psO.tile([128, MTQ, 8], f32, name="S_all")
                S_ps = [S_all[:, mt, :] for mt in range(MTQ)]
                # sums first: Ep is fully materialized, so the row sums land
                # early; the normalize rides the O drains as an activation
                # scale (1/rowsum per partition).
                for mt in range(MTQ):
                    for p in range(GP):
                        nc.tensor.matmul(
                            S_ps[mt][:],
                            Ep[p][:, :, mt * 128 : (mt + 1) * 128],
                            ones_s[:],
                            start=(p == 0),
                            stop=(p == GP - 1),
                            perf_mode=DR,
                        )
                ot_st = [
                    opool.tile([128, MTQ, 512], f32, tag="ot", name=f"ot_st{dh}")
                    for dh in range(2)
                ]
                for mt in range(MTQ):
                    nc.vector.reciprocal(
                        recip_s[:, mt : mt + 1], S_ps[mt][:, 0:1]
                    )
                    # dh0 normalize+store as soon as its sum lands,
                    # overlapping the O1 matmuls (ACT reads PSUM directly)
                    nc.scalar.activation(
                        ot_st[0][:, mt, :], O_ps0[mt][:], AF.Identity,
                        scale=recip_s[:, mt : mt + 1],
                    )
                    nc.sync.dma_start(
                        out.ap()[mt, :, 0:512], ot_st[0][:, mt, :]
                    )
                # mt-major accumulation: O_ps1[mt] closes after its own 32
                # matmuls, so each normalize+store streams out while the PE
                # still accumulates the later mt tiles.
                for mt in range(MTQ):
                    for j in range(NCORES):
                        vt = vt_pre[j]
                        for b in range(2):
                            p = 2 * j + b
                            nc.tensor.matmul(
                                O_ps1[mt][:],
                                Ep[p][:, :, mt * 128 : (mt + 1) * 128],
                                vt[:, 2 * b : 2 * b + 2, 512:1024],
                                start=(p == 0),
                                stop=(p == GP - 1),
                                perf_mode=DR,
                            )
                    nc.vector.tensor_scalar(
                        ot_st[1][:, mt, :], O_ps1[mt][:],
                        recip_s[:, mt : mt + 1], None, ALU.mult,
                    )
                    nc.sync.dma_start(
                        out.ap()[mt, :, 512:1024], ot_st[1][:, mt, :]
                    )
                for j in range(NCORES):
                    vt_pre.pop(j)
            o_stack2.close()
            o_stack.close()


    nc.compile()
    return nc


def make_in_maps(x, attention_mask, learnable_mask, boundary_mask,
                 W_q, b_q, W_k, b_k, W_v, b_v, connection):
    x = np.asarray(x, np.float32)
    mm_full = (np.asarray(attention_mask, np.float32)
               * np.asarray(learnable_mask, np.float32)).astype(np.uint8)
    boundary = np.asarray(boundary_mask, np.float32).reshape(N)

    sel = np.nonzero(boundary > 0.5)[0]
    cap = NCORES * MQ
    assert len(sel) <= cap, (
        f"boundary selects {len(sel)} rows > capacity {cap}"
    )
    sel_padded = np.concatenate(
        [sel, np.zeros(cap - len(sel), np.int64)])

    def w_halves(wt, dt, scale=1.0):  # wt: [D, D], rows = contraction dim
        # -> [2, 128, DC, 512]: [half][p][t][d] = wt[t*128+p][half*512+d]
        a = np.asarray(wt, np.float32).reshape(DC, 128, 2, 512) * scale
        return np.ascontiguousarray(a.transpose(2, 1, 0, 3)).astype(dt)

    wqt_h = w_halves(np.asarray(W_q, np.float32).T, ml_dtypes.float8_e4m3, WSCALE)
    wkt_h = w_halves(np.asarray(W_k, np.float32).T, ml_dtypes.float8_e4m3, WSCALE)
    wvt_h = w_halves(np.asarray(W_v, np.float32).T, ml_dtypes.float8_e4m3, WSCALE)
    cn_h = w_halves(np.asarray(connection, np.float32), ml_dtypes.float8_e4m3,
                    WSCALE)
    bq_h = np.asarray(b_q, np.float32).reshape(DC, 128).T
    bk_h = np.asarray(b_k, np.float32).reshape(DC, 128).T
    bcombo_h = np.concatenate(
        [np.ones((1, 128), np.float32),
         WSCALE * np.asarray(b_v, np.float32).reshape(1, D)],
        axis=1).astype(ml_dtypes.bfloat16)

    def xpose(rows_x):  # [m, D] -> [128, DC, m]: [p][t][i] = x[i][t*128+p]
        m = rows_x.shape[0]
        return np.ascontiguousarray(
            rows_x.T.reshape(DC, 128, m).transpose(1, 0, 2)).astype(
            ml_dtypes.float8_e4m3)

    in_maps = []
    for c in range(NCORES):
        rows = slice(c * M, (c + 1) * M)
        rows_q = sel_padded[c * MQ : (c + 1) * MQ]
        in_maps.append(dict(
            xt=xpose(x[rows]),
            xq=xpose(x[rows_q]),
            wqt=wqt_h, wkt=wkt_h, wvt=wvt_h, cn=cn_h,
            pcombo=np.ascontiguousarray(
                np.concatenate([bq_h, bk_h], axis=1)),
            bcombo=bcombo_h,
            mmh=np.ascontiguousarray(mm_full[rows_q].T).reshape(G, 128, MQ),
            ones8=np.ones((128, 2, 8), dtype=ml_dtypes.float8_e4m3),
        ))
    return in_maps, sel


_cache = {}


def kernel(x, attention_mask, learnable_mask, boundary_mask,
           W_q, b_q, W_k, b_k, W_v, b_v, connection, bias):
    bias_val = float(np.asarray(bias).reshape(-1)[0])
    if bias_val not in _cache:
        _cache[bias_val] = build(bias_val)
    nc = _cache[bias_val]
    in_maps, sel = make_in_maps(x, attention_mask, learnable_mask,
                                boundary_mask, W_q, b_q, W_k, b_k, W_v, b_v,
                                connection)
    res = bass_utils.run_bass_kernel_spmd(nc, in_maps, core_ids=list(range(NCORES)))
    attn = np.concatenate(
        [res.results[c]["out"].reshape(MQ, D) for c in range(NCORES)], axis=0)
    result = np.array(x, np.float32, copy=True)
    result[sel] = attn[: len(sel)]
    return result
